# revision 9
# baseline (speedup 1.0000x reference)
"""Trainium2 Bass kernel for nn_GAT_50861002719407 (2-layer GAT, 8 cores).

Strategy (graph/data parallel, dst-sharded):
 - Nodes padded to 30720 = 8 cores x 30 windows x 128 dsts.
 - Per layer, every core redundantly computes the full node table
   T[node] = [h (256) | alpha_src (8) | alpha_dst (8)] bf16 in its own HBM
   via PE matmuls with a host-extended weight matrix [W | As | Ad]
   (As/Ad fold the per-head attention vectors into the same matmul).
 - Edge phase: edges sorted by destination, sharded by dst window.
   Per 128-edge chunk, an indirect DMA gathers the 272B source rows;
   softmax (exp, no max-sub needed at these magnitudes) and the
   segment-sum aggregation run as one-hot selection matmuls on the PE,
   accumulating [weighted-h | junk | denominator] in PSUM per window.
 - Normalization, bias, ELU on DVE/ACT per window; layer-2 adds the
   final linear head (reduce against broadcast Wl).
 - Two device launches (layer 1, layer 2); the host transposes the
   intermediate activations between launches.
"""
import numpy as np
import ml_dtypes

N = 30000
NP = 30720
E = 480000
F_IN = 128
H = 8
CH = 32
F_H = 256
P = 128
CORES = 8
WPC = 30          # dst windows per core
TW = 272          # table row elements: 256 h + 8 alpha_src + 8 alpha_dst
NEG = 0.2

_progs = {}
_compat_done = False


def _install_compat():
    """Adapt bass/tile output to this walrus build:
    - no EVENT_SEMAPHORE_RANGE_CLEAR support -> nop it
    - only one embedded sync wait/update per instruction -> split onto NoOps
    - provide antenv.axon_hooks so trace=True works."""
    global _compat_done
    if _compat_done:
        return
    _compat_done = True
    import json as _json
    import sys as _sys
    import types as _types
    from concourse import bass
    import concourse.tile as tile
    import concourse.bass_utils as _bu
    import concourse.bass2jax as _b2j
    import bass_rust

    bass.BassGpSimd.sem_clear = lambda self, sem: self.nop()

    def _drain_patched(self, tick_clock, wait_clock):
        from concourse.vector_clock import ScopedClock
        drain_inst = self.nc.sync.drain()
        wait_clock.add_sem_waits(
            drain_inst.ins, ScopedClock({None: tick_clock.global_clock}))
        si = drain_inst.ins.sync_info
        waits = list(si.on_wait) if si else []
        if len(waits) > 1:
            drain_inst.ins.sync_info = bass_rust.SyncInfo(
                on_wait=[waits[0]], on_update=[])
            for w in waits[1:]:
                nop = self.nc.sync.nop()
                nop.ins.sync_info = bass_rust.SyncInfo(on_wait=[w], on_update=[])
        self.nc.all_engine_barrier()
        popped = self.nc._tile_sem_poison_stack.pop()
        assert popped is self._sem_poison
        self.nc.clear_and_free_semaphores(list(self.sems.allocated().values()))
        self.nc.all_engine_barrier()

    tile.TileContext._drain_and_barrier = _drain_patched

    _orig_compile = _bu.compile_bir_kernel

    def _split_sync(bir_json):
        j = _json.loads(bir_json)
        n = 0
        changed = False
        for f in j["functions"]:
            for bb in f["blocks"]:
                out = []
                for ins in bb["instructions"]:
                    si = ins.get("sync_info")
                    waits = si.get("on_wait", []) if si else []
                    ups = si.get("on_update", []) if si else []
                    if len(waits) > 1:
                        changed = True
                        for w in waits[:-1]:
                            n += 1
                            out.append({
                                "debug": ins.get("debug", 0),
                                "engine": ins["engine"], "ins": [],
                                "name": f"syncsplit-w-{n}", "opcode": "NoOp",
                                "outs": [],
                                "sync_info": {"on_update": [], "on_wait": [w]}})
                        si["on_wait"] = [waits[-1]]
                    out.append(ins)
                    if len(ups) > 1:
                        changed = True
                        si["on_update"] = [ups[0]]
                        for u in ups[1:]:
                            n += 1
                            out.append({
                                "debug": ins.get("debug", 0),
                                "engine": ins["engine"], "ins": [],
                                "name": f"syncsplit-u-{n}", "opcode": "NoOp",
                                "outs": [],
                                "sync_info": {"on_update": [u], "on_wait": []}})
                bb["instructions"] = out
        return _json.dumps(j).encode() if changed else bir_json

    def _compat_compile(bir_json, tmpdir, neff_name="file.neff"):
        return _orig_compile(_split_sync(bir_json), tmpdir, neff_name)

    _bu.compile_bir_kernel = _compat_compile
    _b2j.compile_bir_kernel = _compat_compile

    try:
        import antenv.axon_hooks  # noqa: F401
    except ImportError:
        try:
            from trn_agent_boot.trn_boot import _ntff_profile_via_ctypes
            hook = _ntff_profile_via_ctypes('/opt/axon/libaxon_pjrt.so')
        except Exception:
            hook = None
        m = _types.ModuleType('antenv.axon_hooks')
        m.get_axon_ntff_profile_hook = (lambda: hook)
        m.set_axon_ntff_profile_hook = (lambda h: None)
        _sys.modules['antenv.axon_hooks'] = m


def _build_layer(k_in, c_pad, last):
    """Build the bass program for one GAT layer.

    k_in: input feature dim (128 for layer 1, 256 for layer 2).
    last: layer-2 mode (adds linear head; outputs y instead of e1).
    """
    from contextlib import ExitStack
    from concourse import bass, mybir
    import concourse.tile as tile

    bf16 = mybir.dt.bfloat16
    f32 = mybir.dt.float32
    i32 = mybir.dt.int32
    CP = c_pad
    halves = k_in // P

    nc = bass.Bass()
    xT = nc.declare_dram_parameter("xT", [k_in, NP], bf16, isOutput=False)
    Wext = nc.declare_dram_parameter("Wext", [k_in, TW], bf16, isOutput=False)
    idx = nc.declare_dram_parameter("idx", [P, WPC * CP], i32, isOutput=False)
    drel = nc.declare_dram_parameter("drel", [P, WPC * CP], bf16, isOutput=False)
    dwin = nc.declare_dram_parameter("dwin", [P, WPC], i32, isOutput=False)
    Smat = nc.declare_dram_parameter("Smat", [P, WPC * CP * P], bf16,
                                     isOutput=False)
    STmat = nc.declare_dram_parameter("STmat", [P, WPC * CP * P], bf16,
                                      isOutput=False)
    bvec = nc.declare_dram_parameter("bvec", [P, F_H], f32, isOutput=False)
    if last:
        wl = nc.declare_dram_parameter("wl", [P, F_H], f32, isOutput=False)
        blc = nc.declare_dram_parameter("blc", [P, 1], f32, isOutput=False)
        yout = nc.declare_dram_parameter("y", [WPC * P], f32, isOutput=True)
    else:
        eout = nc.declare_dram_parameter("e1", [WPC * P, F_H], bf16, isOutput=True)
    T = nc.dram_tensor("T", [NP, TW], bf16)

    with tile.TileContext(nc) as tc, ExitStack() as ctx:
        cst = ctx.enter_context(tc.tile_pool(name="cst", bufs=1))
        sb = ctx.enter_context(tc.tile_pool(name="sb", bufs=2))
        ps = ctx.enter_context(tc.tile_pool(name="ps", bufs=2, space="PSUM"))

        # load W halves side by side in free dim: [P, halves*TW]
        wx = cst.tile([P, halves * TW], bf16)
        for hh in range(halves):
            nc.sync.dma_start(out=wx[:, hh * TW:(hh + 1) * TW],
                              in_=Wext[hh * P:(hh + 1) * P, :])
        idx_sb = cst.tile([P, WPC * CP], i32)
        nc.sync.dma_start(out=idx_sb[:], in_=idx[:, :])
        drel_sb = cst.tile([P, WPC * CP], bf16)
        nc.sync.dma_start(out=drel_sb[:], in_=drel[:, :])
        dwin_sb = cst.tile([P, WPC], i32)
        nc.sync.dma_start(out=dwin_sb[:], in_=dwin[:, :])
        bvec_sb = cst.tile([P, F_H], f32)
        nc.sync.dma_start(out=bvec_sb[:], in_=bvec[:, :])
        if last:
            wl_sb = cst.tile([P, F_H], f32)
            nc.sync.dma_start(out=wl_sb[:], in_=wl[:, :])
            blc_sb = cst.tile([P, 1], f32)
            nc.sync.dma_start(out=blc_sb[:], in_=blc[:, :])

        # ---- node phase: T[n] = [x@W | x@As | x@Ad] for all NP nodes ----
        GT = NP // P // 8  # tiles per group (30)
        for g in range(8):
            slabs = []
            for hh in range(halves):
                xs = sb.tile([P, GT * P], bf16, tag=f"xslab{hh}")
                nc.sync.dma_start(
                    out=xs[:],
                    in_=xT[hh * P:(hh + 1) * P, g * GT * P:(g + 1) * GT * P])
                slabs.append(xs)
            stage = sb.tile([P, GT, TW], bf16, tag="stage")
            for t in range(GT):
                np_ps = ps.tile([P, TW], f32, tag="nodeps")
                for hh in range(halves):
                    nc.tensor.matmul(
                        np_ps[:],
                        lhsT=slabs[hh][:, t * P:(t + 1) * P],
                        rhs=wx[:, hh * TW:(hh + 1) * TW],
                        start=(hh == 0), stop=(hh == halves - 1))
                nc.vector.tensor_copy(stage[:, t, :], np_ps[:])
            # write table rows for this group: node = g*3840 + t*128 + p
            nc.sync.dma_start(
                out=T[g * GT * P:(g + 1) * GT * P, :].rearrange(
                    "(t p) c -> p t c", p=P),
                in_=stage[:])

        # ---- edge phase ----
        if last:
            ystage = sb.tile([P, WPC], f32, tag="ystage", bufs=1)
        for w in range(WPC):
            adw = sb.tile([P, TW], bf16, tag="adw", bufs=3)
            nc.gpsimd.indirect_dma_start(
                out=adw[:], out_offset=None, in_=T[:, :],
                in_offset=bass.IndirectOffsetOnAxis(
                    ap=dwin_sb[:, w:w + 1], axis=0))
            pw = ps.tile([P, TW], f32, tag="winps")
            Sw = sb.tile([P, CP, P], bf16, tag="Sw", bufs=2)
            nc.sync.dma_start(
                out=Sw[:], in_=Smat[:, w * CP * P:(w + 1) * CP * P])
            STw = sb.tile([P, CP, P], bf16, tag="STw", bufs=2)
            nc.sync.dma_start(
                out=STw[:], in_=STmat[:, w * CP * P:(w + 1) * CP * P])
            for j in range(CP):
                col = w * CP + j
                gat = sb.tile([P, TW], bf16, tag="gat", bufs=16)
                nc.gpsimd.indirect_dma_start(
                    out=gat[:], out_offset=None, in_=T[:, :],
                    in_offset=bass.IndirectOffsetOnAxis(
                        ap=idx_sb[:, col:col + 1], axis=0))
                adp = ps.tile([P, H], f32, tag="adp", bufs=4)
                nc.tensor.matmul(adp[:], lhsT=STw[:, j, :], rhs=adw[:, 264:272],
                                 start=True, stop=True)
                es = sb.tile([P, H], f32, tag="es", bufs=8)
                nc.vector.tensor_add(out=es[:], in0=gat[:, 256:264], in1=adp[:])
                nc.vector.scalar_tensor_tensor(
                    out=es[:], in0=es[:], scalar=NEG, in1=es[:],
                    op0=mybir.AluOpType.mult, op1=mybir.AluOpType.max)
                nc.scalar.activation(es[:], es[:],
                                     mybir.ActivationFunctionType.Exp)
                nc.vector.tensor_copy(gat[:, 264:272], es[:])
                nc.vector.tensor_tensor(
                    out=gat[:, 0:256].rearrange("p (h c) -> p h c", h=H),
                    in0=gat[:, 0:256].rearrange("p (h c) -> p h c", h=H),
                    in1=gat[:, 264:272].to_broadcast([P, H, CH]),
                    op=mybir.AluOpType.mult)
                nc.tensor.matmul(pw[:], lhsT=Sw[:, j, :], rhs=gat[:, 0:TW],
                                 start=(j == 0), stop=(j == CP - 1))
            # window epilogue
            den = sb.tile([P, H], f32, tag="den")
            nc.vector.tensor_scalar_max(out=den[:], in0=pw[:, 264:272],
                                        scalar1=1e-30)
            rec = sb.tile([P, H], f32, tag="rec")
            nc.vector.reciprocal(rec[:], den[:])
            o = sb.tile([P, F_H], f32, tag="o")
            nc.vector.tensor_tensor(
                out=o[:].rearrange("p (h c) -> p h c", h=H),
                in0=pw[:, 0:256].rearrange("p (h c) -> p h c", h=H),
                in1=rec[:].to_broadcast([P, H, CH]),
                op=mybir.AluOpType.mult)
            nc.vector.tensor_add(out=o[:], in0=o[:], in1=bvec_sb[:])
            # ELU(o) = max(o, exp(min(o,0)) - 1)
            u = sb.tile([P, F_H], f32, tag="u")
            nc.vector.tensor_scalar_min(out=u[:], in0=o[:], scalar1=0.0)
            nc.scalar.activation(u[:], u[:], mybir.ActivationFunctionType.Exp)
            nc.vector.scalar_tensor_tensor(
                out=o[:], in0=u[:], scalar=-1.0, in1=o[:],
                op0=mybir.AluOpType.add, op1=mybir.AluOpType.max)
            if last:
                nc.vector.tensor_tensor(out=o[:], in0=o[:], in1=wl_sb[:],
                                        op=mybir.AluOpType.mult)
                yv = sb.tile([P, 1], f32, tag="yv")
                nc.vector.tensor_reduce(out=yv[:], in_=o[:],
                                        axis=mybir.AxisListType.X,
                                        op=mybir.AluOpType.add)
                nc.vector.tensor_scalar_add(out=ystage[:, w:w + 1], in0=yv[:],
                                            scalar1=blc_sb[:, 0:1])
            else:
                eb = sb.tile([P, F_H], bf16, tag="eb", bufs=3)
                nc.vector.tensor_copy(eb[:], o[:])
                nc.sync.dma_start(
                    out=eout[w * P:(w + 1) * P, :], in_=eb[:])
        if last:
            nc.sync.dma_start(
                out=yout[:].rearrange("(w p) -> p w", p=P), in_=ystage[:])
    return nc


def _prep_graph(edge_index):
    ei = np.asarray(edge_index)
    loops = np.arange(N, dtype=ei.dtype)
    src = np.concatenate([ei[0], loops]).astype(np.int64)
    dst = np.concatenate([ei[1], loops]).astype(np.int64)
    order = np.argsort(dst, kind="stable")
    src_s = src[order].astype(np.int32)
    dst_s = dst[order]
    counts = np.bincount(dst_s, minlength=NP)
    nwin = NP // P  # 240
    wcounts = counts.reshape(nwin, P).sum(1)
    c_pad = int(np.ceil(wcounts.max() / P))
    starts = np.zeros(nwin + 1, np.int64)
    np.cumsum(wcounts, out=starts[1:])
    slots = nwin * c_pad * P
    srcs_pad = np.zeros((nwin, c_pad * P), np.int32)
    drel_pad = np.full((nwin, c_pad * P), -1.0, np.float32)
    for wv in range(nwin):
        n_e = wcounts[wv]
        s = starts[wv]
        srcs_pad[wv, :n_e] = src_s[s:s + n_e]
        drel_pad[wv, :n_e] = dst_s[s:s + n_e] - wv * P
    # reshape to [core][128, WPC*CP] with col w*CP+j, partition p = edge j*128+p
    srcs_pc = srcs_pad.reshape(CORES, WPC, c_pad, P).transpose(0, 3, 1, 2)
    drel_pc = drel_pad.reshape(CORES, WPC, c_pad, P).transpose(0, 3, 1, 2)
    idx_arr = np.ascontiguousarray(
        srcs_pc.reshape(CORES, P, WPC * c_pad)).astype(np.int32)
    drel_arr = np.ascontiguousarray(
        drel_pc.reshape(CORES, P, WPC * c_pad)).astype(ml_dtypes.bfloat16)
    # dwin: node ids of each window's 128 dsts
    node_ids = np.arange(NP, dtype=np.int32).reshape(CORES, WPC, P)
    dwin_arr = np.ascontiguousarray(node_ids.transpose(0, 2, 1))  # [c, P, WPC]
    return c_pad, idx_arr, drel_arr, dwin_arr


def _wext(W, a_src, a_dst):
    W = np.asarray(W, np.float32)
    k = W.shape[0]
    Wr = W.reshape(k, H, CH)
    As = np.einsum("khc,hc->kh", Wr, np.asarray(a_src, np.float32))
    Ad = np.einsum("khc,hc->kh", Wr, np.asarray(a_dst, np.float32))
    return np.concatenate([W, As, Ad], axis=1).astype(ml_dtypes.bfloat16)


def _smats(drel_arr, c_pad):
    """Per-core one-hot S [e, d] and S^T [d, e] matrices, bf16.

    Layout [P, WPC*CP*P]: partition p, col (w*CP+j)*P + q."""
    ncols = WPC * c_pad
    S_out = np.zeros((CORES, P, ncols * P), ml_dtypes.bfloat16)
    ST_out = np.zeros((CORES, P, ncols * P), ml_dtypes.bfloat16)
    for c in range(CORES):
        dr = drel_arr[c].astype(np.float32)          # [P, ncols]
        S = (dr[:, :, None] == np.arange(P, dtype=np.float32)[None, None, :])
        Sb = S.astype(ml_dtypes.bfloat16)            # [P, ncols, P]
        S_out[c] = Sb.reshape(P, ncols * P)
        ST_out[c] = np.ascontiguousarray(
            Sb.transpose(2, 1, 0)).reshape(P, ncols * P)
    return S_out, ST_out


LAST_STATS = {}


def kernel(x, edge_index, W1, a1_src, a1_dst, b1, W2, a2_src, a2_dst, b2,
           Wl, bl, _trace=False):
    _install_compat()
    from concourse.bass_utils import run_bass_kernel_spmd

    x = np.asarray(x, np.float32)
    c_pad, idx_arr, drel_arr, dwin_arr = _prep_graph(edge_index)

    key1 = (128, c_pad, False)
    key2 = (256, c_pad, True)
    if key1 not in _progs:
        _progs[key1] = _build_layer(*key1)
    if key2 not in _progs:
        _progs[key2] = _build_layer(*key2)

    smat, stmat = _smats(drel_arr, c_pad)
    w1e = _wext(W1, a1_src, a1_dst)
    w2e = _wext(W2, a2_src, a2_dst)
    b1b = np.tile(np.asarray(b1, np.float32)[None, :], (P, 1))
    b2b = np.tile(np.asarray(b2, np.float32)[None, :], (P, 1))
    wlb = np.tile(np.asarray(Wl, np.float32)[:, 0][None, :], (P, 1))
    blb = np.full((P, 1), float(np.asarray(bl).reshape(-1)[0]), np.float32)

    xT = np.zeros((F_IN, NP), ml_dtypes.bfloat16)
    xT[:, :N] = x.T.astype(ml_dtypes.bfloat16)

    core_ids = list(range(CORES))
    im1 = [{
        "xT": xT, "Wext": w1e, "idx": idx_arr[k], "drel": drel_arr[k],
        "dwin": dwin_arr[k], "Smat": smat[k], "STmat": stmat[k], "bvec": b1b,
    } for k in core_ids]
    r1 = run_bass_kernel_spmd(_progs[key1], im1, core_ids, trace=_trace)
    e1 = np.concatenate([r1.results[k]["e1"] for k in core_ids], axis=0)

    e1T = np.ascontiguousarray(e1.T)  # [256, NP] bf16
    im2 = [{
        "xT": e1T, "Wext": w2e, "idx": idx_arr[k], "drel": drel_arr[k],
        "dwin": dwin_arr[k], "Smat": smat[k], "STmat": stmat[k], "bvec": b2b,
        "wl": wlb, "blc": blb,
    } for k in core_ids]
    r2 = run_bass_kernel_spmd(_progs[key2], im2, core_ids, trace=_trace)
    y = np.concatenate([r2.results[k]["y"] for k in core_ids], axis=0)

    LAST_STATS["exec1_ns"] = r1.exec_time_ns
    LAST_STATS["exec2_ns"] = r2.exec_time_ns
    return y[:N].astype(np.float32)
